# revision 1
# baseline (speedup 1.0000x reference)
"""Conv2D 3x3 stride-1 pad-1 (NCHW) as implicit GEMM on 8 NeuronCores.

Strategy: data-parallel over batch (32 imgs -> 4 per core). The input is
zero-padded on the host to (*, 128, 58, 58) so each image DMAs contiguously
into an SBUF tile [C=128, 58, 58] with input channels on partitions.
Weights are preprocessed host-side to [I=128, (kh kw o)] so each
(tap, ochunk) slice is a ready [K=128, M=128] stationary operand.
Output rows are processed in groups of 8 (moving free dim N = 8*56 = 448),
accumulating the 9 filter taps into one PSUM bank per row-group.

x (4,128,58,58) -> out (4,256,56,56) f32 per core; no collectives.
"""

import os
import sys

import numpy as np

if "/opt/trn_rl_repo" not in sys.path:
    sys.path.insert(0, "/opt/trn_rl_repo")

from concourse import bacc, bass, mybir  # noqa: E402
from concourse.bass_utils import run_bass_kernel_spmd  # noqa: E402
from concourse.tile import TileContext, add_dep_helper  # noqa: E402

N_FULL, CIN, H, W = 32, 128, 56, 56
COUT = 256
KH = KW = 3
NCORES = 8
NPER = N_FULL // NCORES  # 4 images per core
HP, WP = H + 2, W + 2  # 58 x 58 padded
ROWS = 8  # output rows per matmul group
NFREE = ROWS * W  # 448 moving free dim (<= 512 for 4-byte dtypes)
NGROUPS = H // ROWS  # 7
OCH = COUT // 128  # 2 output-channel chunks

# fp32r: full fp32 data streamed through the PE at bf16 rate (free dim >= 256).
MODE = os.environ.get("CONV_MM_MODE", "fp32r")

_CACHE = {}


def _build_conv(mode):
    f32 = mybir.dt.float32
    bf16 = mybir.dt.bfloat16
    if mode == "fp32":
        mm_dt, io_dt = f32, f32
    elif mode == "fp32r":
        mm_dt, io_dt = mybir.dt.float32r, f32
    elif mode in ("bf16", "split3"):
        mm_dt, io_dt = bf16, bf16
    else:
        raise ValueError(mode)

    # Bacc (not raw Bass): its compile pipeline legalizes sync waits --
    # TRN2 instructions carry at most one wait slot.
    nc = bacc.Bacc(None, target_bir_lowering=False)

    if mode == "split3":
        x_names = ["xh", "xl"]
        w_names = ["wh", "wl"]
        # (x_idx, w_idx) matmul passes: hh + hl + lh ~ full fp32 product
        terms = [(0, 0), (0, 1), (1, 0)]
    else:
        x_names = ["x"]
        w_names = ["wt"]
        terms = [(0, 0)]

    x_par = [
        nc.declare_dram_parameter(nm, [NPER, CIN, HP, WP], io_dt, isOutput=False)
        for nm in x_names
    ]
    w_par = [
        nc.declare_dram_parameter(nm, [CIN, KH * KW * COUT], io_dt, isOutput=False)
        for nm in w_names
    ]
    bias_par = nc.declare_dram_parameter("bias", [COUT], f32, isOutput=False)
    out_par = nc.declare_dram_parameter("out", [NPER, COUT, H, W], f32, isOutput=True)
    out_flat = out_par.rearrange("n o h w -> n o (h w)")

    def mmview(ap):
        return ap.bitcast(mm_dt) if mm_dt != io_dt else ap

    nmm_per_psum = KH * KW * len(terms)

    with TileContext(nc) as tc:
        with (
            tc.tile_pool(name="const", bufs=1) as cpool,
            tc.tile_pool(name="xpad", bufs=1) as xpool,
            tc.tile_pool(name="psum", bufs=8, space="PSUM") as ppool,
            tc.tile_pool(name="outp", bufs=4) as opool,
        ):
            # HAM pre-warm: junk matmuls gated only on a prologue memset run
            # during the initial DMA wait so the PE clock gate is at 8/8
            # (2.4 GHz) when the real stream starts. Results never consumed.
            jnk = cpool.tile([128, 512], f32, tag="jnk")
            nc.vector.memset(jnk[:], 1.0)
            jnk_mm = jnk if mm_dt == f32 else jnk.bitcast(mm_dt)
            ps_jnk = ppool.tile([128, NFREE], f32, tag="ps", name="ps")
            for _ in range(8):
                nc.tensor.matmul(
                    ps_jnk[:],
                    jnk_mm[:, 0:128],
                    jnk_mm[:, 0:NFREE],
                    start=True,
                    stop=True,
                )

            # Two padded-x buffers per input tensor (double buffering across
            # images); the zero borders come in with the host-padded DMA.
            xpads = []  # [buf][x_idx] -> tile
            for b in range(2):
                per_buf = []
                for xi in range(len(x_par)):
                    t = xpool.tile(
                        [CIN, HP, WP], mm_dt, tag=f"xpad{b}_{xi}", name="xpad"
                    )
                    per_buf.append(t)
                xpads.append(per_buf)

            # Head loads. Two constraints shape this: a single dma_start
            # tops out ~155 GB/s (vs ~358 GB/s HBM/core) so critical tensors
            # are split across two HW queues, and each issue costs ~0.7us
            # serially on the sync sequencer, so chunks are interleaved
            # x/w/x/w to overlap later issues with earlier transfers.
            # Deferred chunks (oc1 weight halves, image-0 rows 34+) ride
            # behind the first matmul.
            XSPLIT = 34  # padded rows [0,34) cover row-groups 0-3
            w_sb = []
            w3s = []
            for wi, wp in enumerate(w_par):
                t = cpool.tile([CIN, KH * KW * COUT], mm_dt, tag=f"w{wi}", name="w")
                w_sb.append(t)
                w3s.append(
                    (
                        t.rearrange("p (t o) -> p t o", t=KH * KW),
                        mmview(wp[:]).rearrange("p (t o) -> p t o", t=KH * KW),
                    )
                )
            for xi, xp in enumerate(x_par):
                nc.sync.dma_start(
                    out=xpads[0][xi][:, 0:17, :], in_=mmview(xp[0])[:, 0:17, :]
                )
            for t3, w3 in w3s:
                nc.sync.dma_start(out=t3[:, 0:5, 0:128], in_=w3[:, 0:5, 0:128])
            for xi, xp in enumerate(x_par):
                nc.sync.dma_start(
                    out=xpads[0][xi][:, 17:XSPLIT, :],
                    in_=mmview(xp[0])[:, 17:XSPLIT, :],
                )
            for t3, w3 in w3s:
                nc.sync.dma_start(out=t3[:, 5:9, 0:128], in_=w3[:, 5:9, 0:128])
            bias_sb = cpool.tile([128, OCH], f32, tag="bias")
            nc.sync.dma_start(
                out=bias_sb[:], in_=bias_par.rearrange("(a b) -> b a", b=128)
            )
            tail_dmas = []  # released once the first matmul has issued
            for xi, xp in enumerate(x_par):
                d = nc.sync.dma_start(
                    out=xpads[0][xi][:, XSPLIT:HP, :],
                    in_=mmview(xp[0])[:, XSPLIT:HP, :],
                )
                tail_dmas.append(d)
            for t3, w3 in w3s:
                d = nc.sync.dma_start(out=t3[:, :, 128:256], in_=w3[:, :, 128:256])
                tail_dmas.append(d)

            mm_first = None
            mm_oc1_first = None
            x1_dmas = []  # image-1 loads, deferred until the oc1 pass starts
            for n in range(NPER):
                bufs = xpads[n % 2]
                if n >= 1:
                    for xi, xp in enumerate(x_par):
                        # gpsimd queue: slot-reuse waits must not block the
                        # sync queue's output DMAs.
                        d = nc.gpsimd.dma_start(out=bufs[xi][:], in_=mmview(xp[n]))
                        if n == 1:
                            x1_dmas.append(d)
                for oc in range(OCH):
                    psums = [
                        ppool.tile([128, NFREE], f32, tag="ps", name="ps")
                        for _ in range(NGROUPS)
                    ]
                    i_mm = 0
                    for xi, wi in terms:
                        xt = bufs[xi]
                        for tap in range(KH * KW):
                            kh, kw = divmod(tap, KW)
                            lhsT = w_sb[wi][
                                :, tap * COUT + oc * 128 : tap * COUT + oc * 128 + 128
                            ]
                            for g in range(NGROUPS):
                                mm = nc.tensor.matmul(
                                    psums[g][:],
                                    lhsT,
                                    xt[
                                        :,
                                        g * ROWS + kh : g * ROWS + kh + ROWS,
                                        kw : kw + W,
                                    ],
                                    start=(i_mm == 0),
                                    stop=(i_mm == nmm_per_psum - 1),
                                )
                                if n == 0 and i_mm == 0 and g == 0:
                                    if oc == 0:
                                        mm_first = mm
                                    else:
                                        mm_oc1_first = mm
                            i_mm += 1
                    for g in range(NGROUPS):
                        ot = opool.tile([128, NFREE], f32, tag="ot", name="ot")
                        nc.vector.tensor_scalar_add(
                            out=ot[:], in0=psums[g][:], scalar1=bias_sb[:, oc : oc + 1]
                        )
                        nc.sync.dma_start(
                            out=out_flat[
                                n,
                                oc * 128 : (oc + 1) * 128,
                                g * NFREE : (g + 1) * NFREE,
                            ],
                            in_=ot[:],
                        )
            for d in tail_dmas:
                add_dep_helper(
                    d.ins, mm_first.ins, sync=True, reason="defer past first matmul"
                )
            for d in x1_dmas:
                add_dep_helper(
                    d.ins, mm_oc1_first.ins, sync=True, reason="defer image-1 load"
                )
    nc.compile()
    return nc


def _get_nc(mode):
    if mode not in _CACHE:
        _CACHE[mode] = _build_conv(mode)
    return _CACHE[mode]


# test-harness hooks: set TRACE=True before calling kernel() to capture an
# NTFF profile; LAST_RESULTS then holds the BassKernelResults.
TRACE = False
LAST_RESULTS = None


def kernel(x, weight, bias):
    global LAST_RESULTS
    mode = MODE
    x = np.ascontiguousarray(np.asarray(x), dtype=np.float32)
    w = np.ascontiguousarray(np.asarray(weight), dtype=np.float32)
    b = np.ascontiguousarray(np.asarray(bias), dtype=np.float32)
    xp = np.pad(x, ((0, 0), (0, 0), (1, 1), (1, 1)))
    # wt[i, (kh kw o)] = w[o, i, kh, kw]
    wt = np.ascontiguousarray(w.transpose(1, 2, 3, 0).reshape(CIN, KH * KW * COUT))

    if mode in ("fp32", "fp32r"):
        per_core = [
            {"x": xp[c * NPER : (c + 1) * NPER], "wt": wt, "bias": b}
            for c in range(NCORES)
        ]
    else:
        import ml_dtypes

        bfl = ml_dtypes.bfloat16
        if mode == "bf16":
            xh = xp.astype(bfl)
            wth = wt.astype(bfl)
            per_core = [
                {"x": xh[c * NPER : (c + 1) * NPER], "wt": wth, "bias": b}
                for c in range(NCORES)
            ]
        else:  # split3
            xh = xp.astype(bfl)
            xl = (xp - xh.astype(np.float32)).astype(bfl)
            wh = wt.astype(bfl)
            wl = (wt - wh.astype(np.float32)).astype(bfl)
            per_core = [
                {
                    "xh": xh[c * NPER : (c + 1) * NPER],
                    "xl": xl[c * NPER : (c + 1) * NPER],
                    "wh": wh,
                    "wl": wl,
                    "bias": b,
                }
                for c in range(NCORES)
            ]

    kwargs = {}
    if TRACE:
        kwargs = dict(trace=True, trace_cores=[0])
    res = run_bass_kernel_spmd(
        _get_nc(mode), per_core, core_ids=list(range(NCORES)), **kwargs
    )
    LAST_RESULTS = res
    return np.concatenate([r["out"] for r in res.results], axis=0)



# revision 2
# speedup vs baseline: 1.4791x; 1.4791x over previous
"""Conv2D 3x3 stride-1 pad-1 (NCHW) on 8 NeuronCores via 1D Winograd F(2,3).

Strategy: data-parallel over batch (32 imgs -> 4 per core). Winograd F(2,3)
along H cuts tensor-engine work 1.5x vs direct implicit GEMM: for each
output row-pair only 4 winograd components x 3 width-taps = 12 matmul rows
feed 2 output rows (vs 18 direct). All matmul traffic is bf16 (error ~5e-3,
gate 2e-2).

Per image (padded rows 0..57, host-padded, bf16):
  d_a = x[a::2] (28 rows each), a=0..3
  V0 = d0-d2, V1 = d1+d2, V2 = d2-d1, V3 = d1-d3          (DVE, bf16 2x)
  M[i] = sum_kw W'[i,kw]^T V[i][:, ty, kw:kw+56]           (PE, 3-tap PSUM acc)
  evict: m0b = M0+b, m1 = M1, m2 = M2, m3b = M3-b          (ScalarE, ->bf16)
  z0 = (m0b+m1)+m2   -> even out rows                      (DVE)
  z1 = (m1-m2)-m3b   -> odd  out rows                      (DVE)
W'[i,kw][c,o] = sum_kh G[i,kh] w[o,c,kh,kw], G = F(2,3) filter transform,
computed on host in fp32, shipped bf16. Output written bf16, host upcasts.

ty (28 row-pairs) is processed in 4 groups of 7 -> matmul free dim 392,
one PSUM bank per M[i], 4 banks per group, two groups in flight.
"""

import os
import sys

import numpy as np

if "/opt/trn_rl_repo" not in sys.path:
    sys.path.insert(0, "/opt/trn_rl_repo")

from concourse import bacc, bass, mybir  # noqa: E402
from concourse.bass_utils import run_bass_kernel_spmd  # noqa: E402
from concourse.tile import TileContext, add_dep_helper  # noqa: E402

N_FULL, CIN, H, W = 32, 128, 56, 56
COUT = 256
NCORES = 8
NPER = N_FULL // NCORES  # 4 images per core
HP, WP = H + 2, W + 2  # 58 x 58 padded
NI = 4  # winograd components
KWT = 3  # width taps
TY = H // 2  # 28 output row-pairs
GTY = 7  # row-pairs per matmul group
NG = TY // GTY  # 4 groups
NFREE = GTY * W  # 392 moving free dim
OCH = COUT // 128  # 2 output-channel chunks

_CACHE = {}


def _build_conv():
    f32 = mybir.dt.float32
    bf16 = mybir.dt.bfloat16

    nc = bacc.Bacc(None, target_bir_lowering=False)

    x_par = nc.declare_dram_parameter("x", [NPER, CIN, HP, WP], bf16, isOutput=False)
    w_par = nc.declare_dram_parameter("wt", [CIN, NI * KWT * COUT], bf16, isOutput=False)
    bias_par = nc.declare_dram_parameter("bias", [COUT], f32, isOutput=False)
    nbias_par = nc.declare_dram_parameter("nbias", [COUT], f32, isOutput=False)
    out_par = nc.declare_dram_parameter("out", [NPER, COUT, H, W], bf16, isOutput=True)
    out_flat = out_par.rearrange("n o h w -> n o (h w)")
    # dram weight view: [cin, i, kw, o]
    w_dram = w_par.rearrange("p (i k o) -> p (i k) o", i=NI, k=KWT)

    with TileContext(nc) as tc:
        with (
            tc.tile_pool(name="const", bufs=1) as cpool,
            tc.tile_pool(name="xin", bufs=1) as xpool,
            tc.tile_pool(name="vpl", bufs=1) as vpool,
            tc.tile_pool(name="psum", bufs=8, space="PSUM") as ppool,
            tc.tile_pool(name="mev", bufs=8) as mpool,
            tc.tile_pool(name="tu", bufs=4) as tpool,
            tc.tile_pool(name="outp", bufs=4) as opool,
        ):
            # HAM pre-warm: junk matmuls gated only on a prologue memset so
            # the PE clock gate ramps to 8/8 during the initial DMA wait.
            jnk = cpool.tile([128, 512], bf16, tag="jnk")
            nc.vector.memset(jnk[:], 1.0)
            ps_jnk = ppool.tile([128, NFREE], f32, tag="ps", name="ps")
            for _ in range(8):
                nc.tensor.matmul(
                    ps_jnk[:], jnk[:, 0:128], jnk[:, 0:NFREE], start=True, stop=True
                )

            # SBUF tiles: all 4 images + their winograd planes stay resident.
            xts = [
                xpool.tile([CIN, HP, WP], bf16, tag=f"x{n}", name="x") for n in range(NPER)
            ]
            vts = [
                vpool.tile([CIN, NI, TY, HP], bf16, tag=f"v{n}", name="v")
                for n in range(NPER)
            ]
            w_sb = cpool.tile([CIN, NI * KWT, COUT], bf16, tag="w", name="w")
            bias_sb = cpool.tile([128, OCH], f32, tag="bias")
            nbias_sb = cpool.tile([128, OCH], f32, tag="nbias")

            # Head DMAs. Three queues run concurrently (~155 GB/s each):
            #   sync:   weights (oc0 slice first) + biases, later the out tiles
            #   gpsimd: x images, row-halves 0:31
            #   scalar: x images, row-halves 31:58 (ScalarE idle until ~4us)
            XSPL = 31
            nc.sync.dma_start(out=w_sb[:, :, 0:128], in_=w_dram[:, :, 0:128])
            nc.sync.dma_start(
                out=bias_sb[:], in_=bias_par.rearrange("(a b) -> b a", b=128)
            )
            nc.sync.dma_start(
                out=nbias_sb[:], in_=nbias_par.rearrange("(a b) -> b a", b=128)
            )
            nc.sync.dma_start(out=w_sb[:, :, 128:256], in_=w_dram[:, :, 128:256])
            for n in range(NPER):
                nc.gpsimd.dma_start(out=xts[n][:, 0:XSPL, :], in_=x_par[n][:, 0:XSPL, :])
                nc.scalar.dma_start(
                    out=xts[n][:, XSPL:HP, :], in_=x_par[n][:, XSPL:HP, :]
                )

            def v_transform(n, t0, t1):
                """Emit DVE ops computing V planes for image n, ty range [t0,t1)."""
                xv = xts[n].rearrange("p (hh two) w -> p two hh w", two=2)
                v = vts[n]
                # d_a for ty in [t0,t1): d0 = xv[0, t0:t1], d1 = xv[1, t0:t1],
                # d2 = xv[0, t0+1:t1+1], d3 = xv[1, t0+1:t1+1]
                d0 = xv[:, 0, t0:t1, :]
                d1 = xv[:, 1, t0:t1, :]
                d2 = xv[:, 0, t0 + 1 : t1 + 1, :]
                d3 = xv[:, 1, t0 + 1 : t1 + 1, :]
                nc.vector.tensor_sub(v[:, 0, t0:t1, :], d0, d2)
                nc.vector.tensor_add(v[:, 1, t0:t1, :], d1, d2)
                nc.vector.tensor_sub(v[:, 2, t0:t1, :], d2, d1)
                nc.vector.tensor_sub(v[:, 3, t0:t1, :], d1, d3)

            # Image 0 transform in two chunks so matmuls start after the
            # first row-half lands.
            v_transform(0, 0, 14)
            v_transform(0, 14, TY)

            # Deferred V transforms for images 1-3 are emitted interleaved
            # into the previous image's oc=1 g-loop (DVE has slack there).
            for n in range(NPER):
                for oc in range(OCH):
                    for g in range(NG):
                        psums = [
                            ppool.tile([128, NFREE], f32, tag="ps", name="ps")
                            for _ in range(NI)
                        ]
                        for i in range(NI):
                            for kw in range(KWT):
                                nc.tensor.matmul(
                                    psums[i][:],
                                    w_sb[:, i * KWT + kw, oc * 128 : oc * 128 + 128],
                                    vts[n][:, i, g * GTY : (g + 1) * GTY, kw : kw + W],
                                    start=(kw == 0),
                                    stop=(kw == KWT - 1),
                                )
                        # ScalarE evictions (PSUM f32 -> SBUF bf16), bias folded
                        m0b = mpool.tile([128, NFREE], bf16, tag="m", name="m")
                        m1 = mpool.tile([128, NFREE], bf16, tag="m", name="m")
                        m2 = mpool.tile([128, NFREE], bf16, tag="m", name="m")
                        m3b = mpool.tile([128, NFREE], bf16, tag="m", name="m")
                        nc.scalar.add(m0b[:], psums[0][:], bias_sb[:, oc : oc + 1])
                        nc.scalar.copy(m1[:], psums[1][:])
                        nc.scalar.copy(m2[:], psums[2][:])
                        nc.scalar.add(m3b[:], psums[3][:], nbias_sb[:, oc : oc + 1])
                        # DVE output transform into row-interleaved out tile
                        ot = opool.tile([128, GTY, 2, W], bf16, tag="ot", name="ot")
                        t = tpool.tile([128, NFREE], bf16, tag="t", name="t")
                        u = tpool.tile([128, NFREE], bf16, tag="u", name="u")
                        nc.vector.tensor_add(t[:], m0b[:], m1[:])
                        nc.vector.tensor_add(
                            ot[:, :, 0, :], t.rearrange("p (a w) -> p a w", w=W), m2.rearrange("p (a w) -> p a w", w=W)
                        )
                        nc.vector.tensor_sub(u[:], m1[:], m2[:])
                        nc.vector.tensor_sub(
                            ot[:, :, 1, :], u.rearrange("p (a w) -> p a w", w=W), m3b.rearrange("p (a w) -> p a w", w=W)
                        )
                        nc.sync.dma_start(
                            out=out_flat[
                                n,
                                oc * 128 : (oc + 1) * 128,
                                g * (2 * NFREE) : (g + 1) * (2 * NFREE),
                            ],
                            in_=ot.rearrange("p a b w -> p (a b w)"),
                        )
                        # interleave next image's V transform into oc1 stream
                        if oc == 1 and n + 1 < NPER:
                            v_transform(n + 1, g * GTY, (g + 1) * GTY)
    nc.compile()
    return nc


def _get_nc():
    if "wino" not in _CACHE:
        _CACHE["wino"] = _build_conv()
    return _CACHE["wino"]


# test-harness hooks: set TRACE=True before calling kernel() to capture an
# NTFF profile; LAST_RESULTS then holds the BassKernelResults.
TRACE = False
LAST_RESULTS = None
MODE = "wino-bf16"

# F(2,3) filter transform
_G = np.array(
    [[1.0, 0.0, 0.0], [0.5, 0.5, 0.5], [0.5, -0.5, 0.5], [0.0, 0.0, 1.0]],
    dtype=np.float64,
)


def kernel(x, weight, bias):
    global LAST_RESULTS
    import ml_dtypes

    bfl = ml_dtypes.bfloat16

    x = np.ascontiguousarray(np.asarray(x), dtype=np.float32)
    w = np.ascontiguousarray(np.asarray(weight), dtype=np.float32)
    b = np.ascontiguousarray(np.asarray(bias), dtype=np.float32)

    xp = np.pad(x, ((0, 0), (0, 0), (1, 1), (1, 1))).astype(bfl)
    # W'[i, c, kw, o] = sum_kh G[i,kh] w[o,c,kh,kw]  -> layout [c, (i kw o)]
    wp = np.einsum("ik,ockl->iclo", _G, w.astype(np.float64))
    wt = np.ascontiguousarray(
        wp.transpose(1, 0, 2, 3).reshape(CIN, NI * KWT * COUT).astype(np.float32)
    ).astype(bfl)

    per_core = [
        {
            "x": xp[c * NPER : (c + 1) * NPER],
            "wt": wt,
            "bias": b,
            "nbias": -b,
        }
        for c in range(NCORES)
    ]

    kwargs = {}
    if TRACE:
        kwargs = dict(trace=True, trace_cores=[0])
    res = run_bass_kernel_spmd(
        _get_nc(), per_core, core_ids=list(range(NCORES)), **kwargs
    )
    LAST_RESULTS = res
    return np.concatenate([r["out"] for r in res.results], axis=0).astype(np.float32)
